# revision 29
# baseline (speedup 1.0000x reference)
"""AnomalyTransformer forward on 8 trn2 NeuronCores — pure data-parallel over batch.

Layout strategy (per core, 32 batches = 3200 tokens):
  - canonical activations: token-major bf16 tiles A[b] = [100 tok, 512 feat]
  - feature-major copies hT (via PE transpose) feed all weight matmuls
  - attention per batch: scores feature-major q/k -> exp -> PE transpose of
    UNNORMALIZED e (no softmax-stat dependency) -> AV into one PSUM bank ->
    single fused normalize (PSUM read, r broadcast) -> transpose of o ->
    Wo as 4 chunk matmuls; batch-level software pipelining (lookahead 2)
  - FFN: c1 produced feature-major (yT direct, no extra transpose), c2 from
    yT chunks; group-level pipelining overlaps LN (DVE) with next group's PE
  - LN: bn_stats/bn_aggr + rsqrt via bit-trick+Newton on DVE; affine
    scales/biases folded host-side
  - final LN + projection folded into last layer's group loop
"""

import sys
import numpy as np

for _p in ("/opt/trn_rl_repo",):
    if _p not in sys.path:
        sys.path.insert(0, _p)

import ml_dtypes
import concourse.bass as bass
import concourse.bacc as bacc_mod
import concourse.mybir as mybir
from concourse.tile import TileContext
from concourse.bass_utils import run_bass_kernel_spmd

BF16 = ml_dtypes.bfloat16

# model dims
B, L, CIN, COUT = 256, 100, 38, 38
D, H, E, DFF = 512, 8, 3, 512
DH = D // H
NC_CORES = 8
BL = B // NC_CORES          # 32 batches per core
T = BL * L                  # 3200 tokens per core
P = 128
GB = 8                      # batches per attention group
NG = BL // GB               # 4 groups
GT = GB * L                 # 800 tokens per group
KAUG = 3 * CIN + L          # 214 embed contraction rows

f32 = mybir.dt.float32
bf16 = mybir.dt.bfloat16
i32 = mybir.dt.int32
AF = mybir.ActivationFunctionType
ALU = mybir.AluOpType
AX = mybir.AxisListType

MAGIC_P1 = 0x5F3759DF + 1


def _ts(i, n=P):
    return slice(i * n, (i + 1) * n)


def build_nc(trivial_affine=True, zero_bias=True):
    nc = bacc_mod.Bacc()

    # ---- DRAM parameters ------------------------------------------------
    xaugT = nc.declare_dram_parameter("xaugT", [KAUG, T], bf16, isOutput=False)
    wcat = nc.declare_dram_parameter("wcat", [KAUG, D], bf16, isOutput=False)
    wqt = nc.declare_dram_parameter("wqt", [E, 4, P, D], bf16, isOutput=False)
    wkt = nc.declare_dram_parameter("wkt", [E, 4, P, D], bf16, isOutput=False)
    wvt = nc.declare_dram_parameter("wvt", [E, 4, P, D], bf16, isOutput=False)
    wocat = nc.declare_dram_parameter("wocat", [E, 4, P, D], bf16, isOutput=False)
    c1wt = nc.declare_dram_parameter("c1wt", [E, 4, P, D], bf16, isOutput=False)
    c2wt = nc.declare_dram_parameter("c2wt", [E, 4, P, D], bf16, isOutput=False)
    bqf = nc.declare_dram_parameter("bqf", [E, 4, P], f32, isOutput=False)
    bkf = nc.declare_dram_parameter("bkf", [E, 4, P], f32, isOutput=False)
    c1bf = nc.declare_dram_parameter("c1bf", [E, 4, P], f32, isOutput=False)
    btok = nc.declare_dram_parameter("btok", [E, 3, D], bf16, isOutput=False)
    lnsc = nc.declare_dram_parameter("lnsc", [E + 1, 2, 4, P], f32, isOutput=False)
    sbc = nc.declare_dram_parameter("sbc", [E, 2, P, D], bf16, isOutput=False)
    fbc = nc.declare_dram_parameter("fbc", [2, P, D], bf16, isOutput=False)
    maskb = nc.declare_dram_parameter("maskb", [L, 4 * L], bf16, isOutput=False)
    identd = nc.declare_dram_parameter("identd", [P, P], bf16, isOutput=False)
    projt = nc.declare_dram_parameter("projt", [4, P, COUT], bf16, isOutput=False)
    projb = nc.declare_dram_parameter("projb", [COUT], f32, isOutput=False)
    out_d = nc.declare_dram_parameter("out", [COUT, T], f32, isOutput=True)

    with TileContext(nc) as tc:
        with (
            tc.tile_pool(name="const", bufs=1) as cpool,
            tc.tile_pool(name="w", bufs=1) as wpool,
            tc.tile_pool(name="act", bufs=1) as apool,
            tc.tile_pool(name="yp", bufs=2) as ypool,
            tc.tile_pool(name="zs", bufs=9) as zpool,
            tc.tile_pool(name="grp", bufs=2) as gpool,
            tc.tile_pool(name="sc", bufs=3) as spool,
            tc.tile_pool(name="osb", bufs=2) as opool,
            tc.tile_pool(name="pmm", bufs=2, space="PSUM") as pmm,
            tc.tile_pool(name="psS", bufs=2, space="PSUM") as pS,
            tc.tile_pool(name="psT", bufs=2, space="PSUM") as pT,
            tc.tile_pool(name="psO", bufs=2, space="PSUM") as pO,
        ):
            # ---- constants ---------------------------------------------
            idt = cpool.tile([P, P], bf16, tag="ident", name="ident")
            nc.sync.dma_start(out=idt[:, :], in_=identd[:, :])
            mk = cpool.tile([L, 4 * L], bf16, tag="maskb", name="maskb")
            nc.sync.dma_start(out=mk[:, :], in_=maskb[:, :])
            ones1 = cpool.tile([1, P], bf16, tag="ones1", name="ones1")
            nc.vector.memset(ones1[:, :], 1.0)
            pjt = []
            for c in range(4):
                tl = cpool.tile([P, COUT], bf16, tag=f"pjt{c}", name=f"pjt{c}")
                nc.sync.dma_start(out=tl[:, :], in_=projt[c])
                pjt.append(tl)
            pjb = cpool.tile([COUT, 1], f32, tag="pjb", name="pjb")
            nc.sync.dma_start(out=pjb[:, :], in_=projb[:].unsqueeze(1))
            fs = cpool.tile([P, D], bf16, tag="fs", name="fs")
            nc.sync.dma_start(out=fs[:, :], in_=fbc[0])
            fb = cpool.tile([P, D], bf16, tag="fb", name="fb")
            nc.sync.dma_start(out=fb[:, :], in_=fbc[1])
            stats = cpool.tile([L, 2 * BL], f32, tag="stats", name="stats")

            # persistent activation buffers
            A = [apool.tile([L, D], bf16, tag=f"A{b}", name=f"A{b}") for b in range(BL)]
            hT = [apool.tile([P, T], bf16, tag=f"hT{c}", name=f"hT{c}") for c in range(4)]

            # ---- LN helpers --------------------------------------------
            def ln_rsqrt(b0, nb, final_eps=False):
                """rstd for batches [b0, b0+nb) from stats -> returns y tile."""
                w = spool.tile([L, GB], f32, tag="lnw", name="lnw")
                y = spool.tile([L, GB], f32, tag="lny", name="lny")
                t1 = spool.tile([L, GB], f32, tag="lnt", name="lnt")
                vs = stats[:, 2 * b0 + 1: 2 * (b0 + nb): 2]
                if final_eps:
                    # LN(LN(x)) with unit affine == LN with rstd
                    # 1/sqrt(v(1+eps) + eps^2): folds the final LN away
                    nc.vector.tensor_scalar(w[:, 0:nb], vs, 1.0 + 1e-5, 1e-10,
                                            op0=ALU.mult, op1=ALU.add)
                else:
                    nc.vector.tensor_scalar(w[:, 0:nb], vs, 1e-5, None,
                                            op0=ALU.add)
                wi = w[:, 0:nb].bitcast(i32)
                yi = y[:, 0:nb].bitcast(i32)
                ti = t1[:, 0:nb].bitcast(i32)
                nc.vector.tensor_scalar(ti, wi, 1, None,
                                        op0=ALU.logical_shift_right)
                nc.vector.tensor_scalar(ti, ti, -1, None, op0=ALU.bitwise_xor)
                nc.vector.tensor_scalar(yi, ti, MAGIC_P1, None, op0=ALU.add)
                for _ in range(2):
                    nc.vector.tensor_mul(t1[:, 0:nb], y[:, 0:nb], y[:, 0:nb])
                    nc.vector.tensor_mul(t1[:, 0:nb], t1[:, 0:nb], w[:, 0:nb])
                    nc.vector.tensor_scalar(t1[:, 0:nb], t1[:, 0:nb], -0.5, 1.5,
                                            op0=ALU.mult, op1=ALU.add)
                    nc.vector.tensor_mul(y[:, 0:nb], y[:, 0:nb], t1[:, 0:nb])
                return y

            def ln_stats(z, b):
                st6 = spool.tile([L, 6], f32, tag="st6", name="st6", bufs=6)
                nc.vector.bn_stats(st6[:, :], z[:, :])
                nc.vector.bn_aggr(stats[:, 2 * b:2 * b + 2], st6[:, :])

            def ln_apply(zs, b0, nb, final_eps=False):
                y = ln_rsqrt(b0, nb, final_eps)
                for j in range(nb):
                    b = b0 + j
                    eng = nc.gpsimd if j % 2 else nc.vector
                    eng.tensor_scalar(
                        A[b][:, :], zs[b][:, :],
                        stats[:, 2 * b:2 * b + 1], y[:, j:j + 1],
                        op0=ALU.subtract, op1=ALU.mult)

            def transpose_group(src_tiles, b0, nb, dst, scale_ch):
                """dst[c][:, (b0..b0+nb)*L] = src[b][:, chunk c].T (*scale)."""
                for c in range(4):
                    ps = pmm.tile([P, 2 * D], bf16, tag="mm", name="mm")
                    for j in range(nb):
                        nc.tensor.matmul(ps[:, j * L:(j + 1) * L],
                                         src_tiles[b0 + j][:, _ts(c)],
                                         idt[0:L, 0:L], is_transpose=True,
                                         start=(j == 0), stop=(j == nb - 1))
                    w = nb * L
                    dst_ap = dst[c][:, b0 * L:b0 * L + w]
                    if scale_ch is not None:
                        nc.scalar.activation(dst_ap, ps[:, 0:w], AF.Identity,
                                             bias=0.0,
                                             scale=scale_ch[c][:, 0:1])
                    else:
                        if c % 2 == 0:
                            nc.scalar.activation(dst_ap, ps[:, 0:w], AF.Identity)
                        else:
                            nc.vector.tensor_copy(dst_ap, ps[:, 0:w])

            # ---- embed: h0 = xaug @ wcat  (PE folded into wcat rows) ----
            wc0 = wpool.tile([P, D], bf16, tag="wq0", name="wc0")
            nc.sync.dma_start(out=wc0[:, :], in_=wcat[0:P, :])
            wc1 = wpool.tile([KAUG - P, D], bf16, tag="wq1", name="wc1")
            nc.sync.dma_start(out=wc1[:, :], in_=wcat[P:KAUG, :])
            xa0 = apool.tile([P, T], bf16, tag="hT0", name="xa0")
            xa1 = apool.tile([KAUG - P, T], bf16, tag="hT1", name="xa1")
            for g in range(NG):
                gc = slice(g * GT, (g + 1) * GT)
                nc.sync.dma_start(out=xa0[:, gc], in_=xaugT[0:P, gc])
                nc.sync.dma_start(out=xa1[:, gc], in_=xaugT[P:KAUG, gc])
            for b in range(BL):
                bc = slice(b * L, (b + 1) * L)
                ps = pmm.tile([L, D], f32, tag="mm", name="mm")
                nc.tensor.matmul(ps[:, :], xa0[:, bc], wc0[:, :],
                                 start=True, stop=False)
                nc.tensor.matmul(ps[:, :], xa1[:, bc], wc1[:, :],
                                 start=False, stop=True)
                if b % 2 == 0:
                    nc.vector.tensor_copy(A[b][:, :], ps[:, :])
                else:
                    nc.scalar.activation(A[b][:, :], ps[:, :], AF.Identity)
            # hT0/hT1 alias xa0/xa1 (tag ring); the slot handback requires all
            # xa reads emitted first, so transposes must follow the full embed
            for bg in range(BL // 4):
                transpose_group(A, bg * 4, 4, hT, None)

            # ---- attention group ---------------------------------------
            def attn_group(g, Wq, Wk, Wv, Wo, bq, bk, bt, sinb):
                """QKV + per-batch pipelined attention; returns zs dict."""
                g0 = g * GT
                b0 = g * GB
                Qg = [gpool.tile([P, GT], bf16, tag=f"qg{c}", name=f"qg{c}")
                      for c in range(4)]
                Kg = [gpool.tile([P, GT], bf16, tag=f"kg{c}", name=f"kg{c}")
                      for c in range(4)]
                for co in range(4):
                    for hh in range(2):
                        colv = slice(g0 + hh * 400, g0 + (hh + 1) * 400)
                        ps = pmm.tile([P, D], f32, tag="mm", name="mm")
                        for ci in range(4):
                            nc.tensor.matmul(ps[:, 0:400],
                                             Wq[ci][:, _ts(co)],
                                             hT[ci][:, colv],
                                             start=(ci == 0), stop=(ci == 3))
                        if zero_bias:
                            nc.scalar.activation(
                                Qg[co][:, hh * 400:(hh + 1) * 400],
                                ps[:, 0:400], AF.Identity)
                        else:
                            nc.scalar.activation(
                                Qg[co][:, hh * 400:(hh + 1) * 400],
                                ps[:, 0:400], AF.Identity,
                                bias=bq[co][:, 0:1])
                        ps = pmm.tile([P, D], f32, tag="mm", name="mm")
                        for ci in range(4):
                            nc.tensor.matmul(ps[:, 0:400],
                                             Wk[ci][:, _ts(co)],
                                             hT[ci][:, colv],
                                             start=(ci == 0), stop=(ci == 3))
                        if zero_bias:
                            if co % 2:
                                nc.scalar.activation(
                                    Kg[co][:, hh * 400:(hh + 1) * 400],
                                    ps[:, 0:400], AF.Identity)
                            else:
                                nc.vector.tensor_copy(
                                    Kg[co][:, hh * 400:(hh + 1) * 400],
                                    ps[:, 0:400])
                        else:
                            nc.vector.tensor_scalar(
                                Kg[co][:, hh * 400:(hh + 1) * 400],
                                ps[:, 0:400], bk[co][:, 0:1], None,
                                op0=ALU.add)
                Qg2 = [gpool.tile([64, GT], bf16, tag=f"qh{c}", name=f"qh{c}",
                                  bufs=1)
                       for c in range(4)]
                Kg2 = [gpool.tile([64, GT], bf16, tag=f"kh{c}", name=f"kh{c}",
                                  bufs=1)
                       for c in range(4)]
                for c in range(4):
                    nc.sync.dma_start(out=Qg2[c][:, :], in_=Qg[c][64:128, :])
                    nc.sync.dma_start(out=Kg2[c][:, :], in_=Kg[c][64:128, :])
                Vg = [gpool.tile([L, D], bf16, tag=f"vg{b}", name=f"vg{b}")
                      for b in range(GB)]
                for b in range(GB):
                    ps = pmm.tile([L, D], f32, tag="mm", name="mm")
                    for ci in range(4):
                        nc.tensor.matmul(ps[:, :],
                                         hT[ci][:, g0 + b * L:g0 + (b + 1) * L],
                                         Wv[ci][:, :],
                                         start=(ci == 0), stop=(ci == 3))
                    if b % 2 == 0:
                        nc.vector.tensor_copy(Vg[b][:, :], ps[:, :])
                    else:
                        nc.scalar.activation(Vg[b][:, :], ps[:, :], AF.Identity)

                e_t = {}
                r8_t = {}
                eT_t = {}
                oc_t = {}
                ocT_t = {}
                zs = {}

                def s_scores(b):
                    """scores + mask + exp; row-sum reciprocal on DVE."""
                    e = spool.tile([L, H * L], bf16, tag="e", name="e", bufs=3)
                    for half in range(2):
                        psS = pS.tile([L, D], f32, tag="S", name="S")
                        for hj in range(4):
                            h = half * 4 + hj
                            c = h // 2
                            qsrc = Qg2[c] if h % 2 else Qg[c]
                            ksrc = Kg2[c] if h % 2 else Kg[c]
                            nc.tensor.matmul(
                                psS[:, hj * L:(hj + 1) * L],
                                qsrc[0:64, b * L:(b + 1) * L],
                                ksrc[0:64, b * L:(b + 1) * L],
                                start=(hj == 0), stop=False)
                        nc.tensor.matmul(psS[:, 0:4 * L], idt[0:L, 0:L],
                                         mk[:, :], start=False, stop=True)
                        nc.scalar.activation(
                            e[:, half * 4 * L:(half + 1) * 4 * L],
                            psS[:, 0:4 * L], AF.Exp)
                    s8 = spool.tile([L, H], f32, tag="s8", name="s8", bufs=4)
                    nc.vector.reduce_sum(
                        out=s8[:, :],
                        in_=e[:, :].rearrange("p (h m) -> p h m", h=H),
                        axis=AX.X)
                    r8 = spool.tile([L, H], f32, tag="r8", name="r8", bufs=4)
                    nc.vector.reciprocal(r8[:, :], s8[:, :])
                    e_t[b] = e
                    r8_t[b] = r8

                def s_etrans(b):
                    """PE transpose of UNNORMALIZED e (no softmax-stat dep)."""
                    e = e_t.pop(b)
                    eTb = spool.tile([L, H * L], bf16, tag="eT", name="eTb",
                                     bufs=3)
                    psA = pT.tile([L, 2 * D], bf16, tag="aTp", name="aTp")
                    for h in range(H):
                        nc.tensor.matmul(psA[:, h * L:(h + 1) * L],
                                         e[:, h * L:(h + 1) * L],
                                         idt[0:L, 0:L], is_transpose=True,
                                         start=(h == 0), stop=(h == 7))
                    nc.scalar.activation(eTb[:, :], psA[:, 0:8 * L],
                                         AF.Identity)
                    eT_t[b] = eTb

                def s_av(b):
                    """AV into one PSUM bank + fused normalize (DVE)."""
                    eTb = eT_t.pop(b)
                    psO = pO.tile([L, D], f32, tag="O", name="O")
                    for h in range(H):
                        nc.tensor.matmul(
                            psO[:, h * DH:(h + 1) * DH],
                            eTb[0:L, h * L:(h + 1) * L],
                            Vg[b][:, h * DH:(h + 1) * DH],
                            start=(h == 0), stop=(h == 7))
                    oc = spool.tile([L, D], bf16, tag="oc", name="oc", bufs=3)
                    r8 = r8_t.pop(b)
                    nc.vector.tensor_mul(
                        oc[:, :].rearrange("p (h d) -> p h d", h=H),
                        psO[:, :].rearrange("p (h d) -> p h d", h=H),
                        r8[:, :].to_broadcast((L, H, DH)))
                    oc_t[b] = oc

                def s_octrans(b):
                    """transpose oc into feature-major chunks."""
                    oc = oc_t.pop(b)
                    psB = pmm.tile([P, 2 * D], bf16, tag="mm", name="mm")
                    for c in range(4):
                        nc.tensor.matmul(psB[:, c * L:(c + 1) * L],
                                         oc[:, _ts(c)],
                                         idt[0:L, 0:L], is_transpose=True,
                                         start=(c == 0), stop=(c == 3))
                    ocTb = spool.tile([P, 4 * L], bf16, tag="ocT", name="ocTb",
                                      bufs=3)
                    nc.scalar.activation(ocTb[:, :], psB[:, 0:4 * L],
                                         AF.Identity)
                    ocT_t[b] = ocTb

                def s_wo(b):
                    """Wo chunk matmuls, residual, ln stats."""
                    bg = b0 + b
                    ocTb = ocT_t.pop(b)
                    psZ = pmm.tile([L, D], f32, tag="mm", name="mm")
                    for c in range(4):
                        nc.tensor.matmul(psZ[:, :],
                                         ocTb[:, c * L:(c + 1) * L],
                                         Wo[c][:, :],
                                         start=(c == 0),
                                         stop=(c == 3 and zero_bias))
                    if not zero_bias:
                        nc.tensor.matmul(psZ[:, :], ones1[:, 0:L],
                                         bt[0][:, :], start=False, stop=True)
                    if trivial_affine:
                        rsrc = A[bg]
                    else:
                        rsrc = spool.tile([L, D], bf16, tag="r", name="r")
                        nc.gpsimd.tensor_mul(rsrc[:, :], A[bg][:, :],
                                             sinb[0:L, :])
                    z = zpool.tile([L, D], bf16, tag="za", name="z")
                    nc.vector.tensor_add(z[:, :], psZ[:, :], rsrc[:, :])
                    zs[bg] = z
                    ln_stats(z, bg)

                stages = (s_scores, s_etrans, s_av, s_octrans, s_wo)
                NS = len(stages)
                for step in range(GB + NS - 1):
                    for d, fn in enumerate(stages):
                        b = step - d
                        if 0 <= b < GB:
                            fn(b)
                return zs

            # ---- FFN pieces (c1 feature-major; c2 + LN2; T2 separate) --
            def c1_group(g, C1, c1b):
                g0 = g * GT
                yTg = [ypool.tile([P, GT], bf16, tag=f"yt{c}", name=f"yt{c}")
                       for c in range(4)]
                for co in range(4):
                    for hh in range(2):
                        colv = slice(g0 + hh * 400, g0 + (hh + 1) * 400)
                        ps = pmm.tile([P, D], f32, tag="mm", name="mm")
                        for ci in range(4):
                            nc.tensor.matmul(ps[:, 0:400],
                                             C1[ci][:, _ts(co)],
                                             hT[ci][:, colv],
                                             start=(ci == 0), stop=(ci == 3))
                        if zero_bias:
                            nc.scalar.activation(
                                yTg[co][:, hh * 400:(hh + 1) * 400],
                                ps[:, 0:400], AF.Gelu)
                        else:
                            nc.scalar.activation(
                                yTg[co][:, hh * 400:(hh + 1) * 400],
                                ps[:, 0:400], AF.Gelu,
                                bias=c1b[co][:, 0:1])
                return yTg

            def c2_half(g, half, yTg, C2, bt, s1b, fuse_final):
                b0 = g * GB
                zs = {}
                for j in range(4):
                    b = half * 4 + j
                    bg = b0 + b
                    ps2 = pmm.tile([L, D], f32, tag="mm", name="mm")
                    for ci in range(4):
                        nc.tensor.matmul(ps2[:, :],
                                         yTg[ci][:, b * L:(b + 1) * L],
                                         C2[ci][:, :],
                                         start=(ci == 0),
                                         stop=(ci == 3 and zero_bias))
                    if not zero_bias:
                        nc.tensor.matmul(ps2[:, :], ones1[:, 0:L],
                                         bt[2][:, :], start=False,
                                         stop=True)
                    if trivial_affine:
                        rsrc = A[bg]
                    else:
                        rsrc = spool.tile([L, D], bf16, tag="r", name="r")
                        nc.gpsimd.tensor_mul(rsrc[:, :], A[bg][:, :],
                                             s1b[0:L, :])
                    z = zpool.tile([L, D], bf16, tag="zf", name="z",
                                   bufs=6)
                    nc.vector.tensor_add(z[:, :], ps2[:, :], rsrc[:, :])
                    zs[bg] = z
                    ln_stats(z, bg)
                ln_apply(zs, b0 + half * 4, 4, final_eps=fuse_final)

            def t2_group(g, scale_ch):
                transpose_group(A, g * GB, 4, hT, scale_ch)
                transpose_group(A, g * GB + 4, 4, hT, scale_ch)

            # ---- projection for one group ------------------------------
            def proj_group(g):
                for hh in range(2):
                    n0 = g * GT + hh * 400
                    ps = pmm.tile([P, D], f32, tag="mm", name="mm")
                    for ci in range(4):
                        nc.tensor.matmul(ps[0:COUT, 0:400], pjt[ci][:, :],
                                         hT[ci][:, n0:n0 + 400],
                                         start=(ci == 0), stop=(ci == 3))
                    osb = opool.tile([COUT, 400], f32, tag="osb", name="osb")
                    if zero_bias:
                        nc.vector.tensor_copy(osb[:, :], ps[0:COUT, 0:400])
                    else:
                        nc.vector.tensor_scalar(osb[:, :], ps[0:COUT, 0:400],
                                                pjb[:, 0:1], None, op0=ALU.add)
                    nc.sync.dma_start(out=out_d[:, n0:n0 + 400], in_=osb[:, :])

            # ---- final LN + projection for one group (non-trivial) -----
            def final_group(g):
                b0 = g * GB
                zs = {}
                for j in range(GB):
                    b = b0 + j
                    if trivial_affine:
                        z = A[b]
                    else:
                        r = spool.tile([L, D], bf16, tag="r", name="r")
                        nc.gpsimd.tensor_mul(r[:, :], A[b][:, :], fs[0:L, :])
                        z = zpool.tile([L, D], bf16, tag="zf", name="z",
                                       bufs=6)
                        nc.vector.tensor_add(z[:, :], r[:, :], fb[0:L, :])
                    zs[b] = z
                    ln_stats(z, b)
                ln_apply(zs, b0, GB)
                transpose_group(A, b0, 4, hT, None)
                transpose_group(A, b0 + 4, 4, hT, None)
                proj_group(g)

            # ---- layers -------------------------------------------------
            for l in range(E):
                last_layer = (l == E - 1)
                Wq, Wk, Wv, Wo, C1, C2 = [], [], [], [], [], []
                for c in range(4):
                    for lst, nm, drm in ((Wq, "wq", wqt), (Wk, "wk", wkt),
                                         (Wv, "wv", wvt), (Wo, "wo", wocat),
                                         (C1, "c1", c1wt), (C2, "c2", c2wt)):
                        tl = wpool.tile([P, D], bf16, tag=f"{nm}{c}",
                                        name=f"{nm}{c}")
                        nc.sync.dma_start(out=tl[:, :], in_=drm[l, c])
                        lst.append(tl)
                bq, bk, c1b, sin_ch, s1_ch, sn_ch = [], [], [], [], [], []
                for c in range(4):
                    tl = wpool.tile([P, 1], f32, tag=f"bq{c}", name=f"bq{c}")
                    nc.sync.dma_start(out=tl[:, :], in_=bqf[l, c].unsqueeze(1))
                    bq.append(tl)
                    tl = wpool.tile([P, 1], f32, tag=f"bk{c}", name=f"bk{c}")
                    nc.sync.dma_start(out=tl[:, :], in_=bkf[l, c].unsqueeze(1))
                    bk.append(tl)
                    tl = wpool.tile([P, 1], f32, tag=f"cb{c}", name=f"cb{c}")
                    nc.sync.dma_start(out=tl[:, :], in_=c1bf[l, c].unsqueeze(1))
                    c1b.append(tl)
                    tl = wpool.tile([P, 1], f32, tag=f"s1{c}", name=f"s1{c}")
                    nc.sync.dma_start(out=tl[:, :], in_=lnsc[l, 1, c].unsqueeze(1))
                    s1_ch.append(tl)
                    tl = wpool.tile([P, 1], f32, tag=f"sn{c}", name=f"sn{c}")
                    nc.sync.dma_start(out=tl[:, :],
                                      in_=lnsc[l + 1, 0, c].unsqueeze(1))
                    sn_ch.append(tl)
                bt = []
                for i in range(3):
                    tl = wpool.tile([1, D], bf16, tag=f"bt{i}", name=f"bt{i}")
                    nc.sync.dma_start(out=tl[:, :], in_=btok[l, i].unsqueeze(0))
                    bt.append(tl)
                sinb = wpool.tile([P, D], bf16, tag="sinb", name="sinb")
                nc.sync.dma_start(out=sinb[:, :], in_=sbc[l, 0])
                s1b = wpool.tile([P, D], bf16, tag="s1b", name="s1b")
                nc.sync.dma_start(out=s1b[:, :], in_=sbc[l, 1])

                fuse_final = last_layer and trivial_affine
                t1_sc = None if trivial_affine else s1_ch
                t2_sc = (None if (trivial_affine or last_layer) else sn_ch)

                # phase A: attention over all groups; LN1 of group g-1 runs
                # on DVE under group g's matmuls, its transpose right after
                zs_attn = [None] * NG
                for g in range(NG):
                    if g >= 1:
                        ln_apply(zs_attn[g - 1], (g - 1) * GB, GB)
                    zs_attn[g] = attn_group(g, Wq, Wk, Wv, Wo, bq, bk, bt,
                                            sinb)
                    if g >= 1:
                        transpose_group(A, (g - 1) * GB, 4, hT, t1_sc)
                        transpose_group(A, (g - 1) * GB + 4, 4, hT, t1_sc)
                ln_apply(zs_attn[NG - 1], (NG - 1) * GB, GB)

                # phase B: FFN over all groups (single act-table switch);
                # T1 of the last group is hidden under c1 of group 0
                yT_g = [None] * NG
                yT_g[0] = c1_group(0, C1, c1b)
                transpose_group(A, (NG - 1) * GB, 4, hT, t1_sc)
                transpose_group(A, (NG - 1) * GB + 4, 4, hT, t1_sc)
                for g in range(NG):
                    c2_half(g, 0, yT_g[g], C2, bt, s1b, fuse_final)
                    c2_half(g, 1, yT_g[g], C2, bt, s1b, fuse_final)
                    if g + 1 < NG:
                        yT_g[g + 1] = c1_group(g + 1, C1, c1b)
                    if not last_layer or fuse_final:
                        t2_group(g, None if fuse_final else t2_sc)
                    if last_layer:
                        if trivial_affine:
                            proj_group(g)
                        else:
                            final_group(g)

    nc.compile()
    return nc


# ---------------------------------------------------------------------------
# host side
# ---------------------------------------------------------------------------

def _pos_encoding():
    pos = np.arange(L)[:, None].astype(np.float32)
    div = np.exp(np.arange(0, D, 2).astype(np.float32) * (-np.log(10000.0) / D))
    pe = np.zeros((L, D), dtype=np.float32)
    pe[:, 0::2] = np.sin(pos * div)
    pe[:, 1::2] = np.cos(pos * div)
    return pe


def _chunk4(mT):
    """[D, N] -> [4, 128, N]"""
    return np.ascontiguousarray(mT.reshape(4, P, -1))


_NC = None
_NC_FLAGS = None


def _get_nc(trivial_affine=True, zero_bias=True):
    global _NC, _NC_FLAGS
    if _NC is None or _NC_FLAGS != (trivial_affine, zero_bias):
        _NC = build_nc(trivial_affine, zero_bias)
        _NC_FLAGS = (trivial_affine, zero_bias)
    return _NC


def is_trivial_affine(inputs):
    return (np.all(np.asarray(inputs["ln1s"]) == 1.0)
            and np.all(np.asarray(inputs["ln2s"]) == 1.0)
            and np.all(np.asarray(inputs["ln2b"])[E - 1] == 0.0))


def prepare_maps(inputs):
    inp = {k: np.asarray(v) for k, v in inputs.items()}
    x = inp["x"].astype(np.float32)
    emb_w = inp["emb_w"].astype(np.float32)
    mask = inp["mask"].astype(np.float32)

    Wq, bqa = inp["Wq"], inp["bq"]
    Wk, bka = inp["Wk"], inp["bk"]
    Wv, bva = inp["Wv"], inp["bv"]
    Wo, boa = inp["Wo"], inp["bo"]
    c1w, c1b = inp["c1w"], inp["c1b"]
    c2w, c2b = inp["c2w"], inp["c2b"]
    ln1s, ln1b = inp["ln1s"], inp["ln1b"]
    ln2s, ln2b = inp["ln2s"], inp["ln2b"]
    lnfs, lnfb = inp["lnfs"], inp["lnfb"]
    proj_w, proj_b = inp["proj_w"], inp["proj_b"]

    scale = 1.0 / np.sqrt(DH)

    wqt = np.stack([_chunk4(Wq[l].T * scale) for l in range(E)]).astype(BF16)
    wkt = np.stack([_chunk4(Wk[l].T) for l in range(E)]).astype(BF16)
    wvt = np.stack([_chunk4(Wv[l].T) for l in range(E)]).astype(BF16)
    wocat = np.stack([_chunk4(Wo[l].T) for l in range(E)]).astype(BF16)
    c1wt = np.stack([_chunk4(c1w[l].T) for l in range(E)]).astype(BF16)
    c2wt = np.stack([_chunk4(c2w[l].T) for l in range(E)]).astype(BF16)

    bqf = np.zeros((E, 4, P), np.float32)
    bkf = np.zeros((E, 4, P), np.float32)
    c1bf = np.zeros((E, 4, P), np.float32)
    btok = np.zeros((E, 3, D), np.float32)
    lnsc = np.zeros((E + 1, 2, 4, P), np.float32)
    lnsc[:, :] = 1.0
    sbc = np.zeros((E, 2, P, D), np.float32)
    for l in range(E):
        b_in = ln2b[l - 1] if l > 0 else np.zeros(D, np.float32)
        s_in = ln2s[l - 1] if l > 0 else np.ones(D, np.float32)
        bq_eff = (bqa[l] + b_in @ Wq[l].T) * scale
        bk_eff = bka[l] + b_in @ Wk[l].T
        bv_eff = bva[l] + b_in @ Wv[l].T
        bo_eff = boa[l] + bv_eff @ Wo[l].T + b_in
        c1b_eff = c1b[l] + ln1b[l] @ c1w[l].T
        c2b_eff = c2b[l] + ln1b[l]
        bqf[l] = bq_eff.reshape(4, P)
        bkf[l] = bk_eff.reshape(4, P)
        c1bf[l] = c1b_eff.reshape(4, P)
        btok[l, 0] = bo_eff
        btok[l, 1] = c1b_eff
        btok[l, 2] = c2b_eff
        lnsc[l, 0] = s_in.reshape(4, P)
        lnsc[l, 1] = ln1s[l].reshape(4, P)
        sbc[l, 0] = np.tile(s_in[None, :], (P, 1))
        sbc[l, 1] = np.tile(ln1s[l][None, :], (P, 1))

    fbc = np.stack([np.tile(ln2s[E - 1][None, :], (P, 1)),
                    np.tile(ln2b[E - 1][None, :], (P, 1))]).astype(BF16)

    projw_eff = proj_w * lnfs[None, :]
    projb_eff = proj_b + lnfb @ proj_w.T
    projt = np.ascontiguousarray(
        projw_eff.T.reshape(4, P, COUT)).astype(BF16)

    maskb_np = np.tile(-30.0 * (1.0 - mask), (1, 4)).astype(BF16)
    ident = np.eye(P, dtype=np.float32).astype(BF16)
    wcat = np.concatenate([emb_w[:, :, 0].T, emb_w[:, :, 1].T,
                           emb_w[:, :, 2].T, _pos_encoding()], axis=0)

    shared = dict(
        wcat=wcat.astype(BF16), wqt=wqt, wkt=wkt, wvt=wvt, wocat=wocat,
        c1wt=c1wt, c2wt=c2wt, bqf=bqf, bkf=bkf, c1bf=c1bf,
        btok=btok.astype(BF16), lnsc=lnsc, sbc=sbc.astype(BF16),
        fbc=fbc, maskb=maskb_np, identd=ident, projt=projt,
        projb=projb_eff.astype(np.float32),
    )

    # per-core augmented input, feature-major [214, 3200]
    oh = np.eye(L, dtype=np.float32)
    in_maps = []
    for ci in range(NC_CORES):
        xs = x[ci * BL:(ci + 1) * BL]                      # [32, 100, 38]
        xp = np.concatenate([xs[:, -1:], xs, xs[:, :1]], axis=1)  # [32,102,38]
        feats = [xp[:, w:w + L, :] for w in range(3)]      # each [32,100,38]
        ohb = np.broadcast_to(oh[None], (BL, L, L))
        xaug = np.concatenate(feats + [ohb], axis=2)       # [32,100,214]
        xaugT = np.ascontiguousarray(
            xaug.reshape(T, KAUG).T).astype(BF16)          # [214, 3200]
        m = dict(shared)
        m["xaugT"] = xaugT
        in_maps.append(m)
    return in_maps


def is_zero_bias(inputs):
    i = {k: np.asarray(v) for k, v in inputs.items()}
    zb = True
    for l in range(E):
        b_in = i["ln2b"][l - 1] if l > 0 else np.zeros(D, np.float32)
        zb &= bool(np.all(i["bq"][l] + b_in @ i["Wq"][l].T == 0))
        zb &= bool(np.all(i["bk"][l] + b_in @ i["Wk"][l].T == 0))
        bv_eff = i["bv"][l] + b_in @ i["Wv"][l].T
        zb &= bool(np.all(i["bo"][l] + bv_eff @ i["Wo"][l].T + b_in == 0))
        zb &= bool(np.all(i["c1b"][l] + i["ln1b"][l] @ i["c1w"][l].T == 0))
        zb &= bool(np.all(i["c2b"][l] + i["ln1b"][l] == 0))
    zb &= bool(np.all(i["proj_b"] + i["lnfb"] @ i["proj_w"].T == 0))
    return zb


def run(inputs, **kw):
    nc = _get_nc(is_trivial_affine(inputs), is_zero_bias(inputs))
    in_maps = prepare_maps(inputs)
    res = run_bass_kernel_spmd(nc, in_maps, core_ids=list(range(NC_CORES)), **kw)
    outs = []
    for ci in range(NC_CORES):
        o = np.asarray(res.results[ci]["out"], np.float32)  # [38, 3200]
        outs.append(o.T.reshape(BL, L, COUT))
    full = np.concatenate(outs, axis=0)
    return full, res


def kernel(**inputs):
    full, _ = run(inputs)
    return full.astype(np.float32)


def bench(inputs, iters=6):
    """Steady-state wall timing of the sharded jitted executable."""
    import time
    import jax
    from jax.sharding import Mesh, PartitionSpec
    from jax.experimental.shard_map import shard_map
    from concourse import bass2jax, mybir
    from concourse.bass2jax import _bass_exec_p, install_neuronx_cc_hook, partition_id_tensor

    nc = _get_nc(is_trivial_affine(inputs), is_zero_bias(inputs))
    in_maps = prepare_maps(inputs)
    install_neuronx_cc_hook()
    partition_name = nc.partition_id_tensor.name if nc.partition_id_tensor else None
    in_names, out_names, out_avals, zero_outs = [], [], [], []
    for alloc in nc.m.functions[0].allocations:
        if not isinstance(alloc, mybir.MemoryLocationSet):
            continue
        name = alloc.memorylocations[0].name
        if alloc.kind == "ExternalInput":
            if name != partition_name:
                in_names.append(name)
        elif alloc.kind == "ExternalOutput":
            out_names.append(name)
            shape = tuple(alloc.tensor_shape)
            dtype = mybir.dt.np(alloc.dtype)
            out_avals.append(jax.core.ShapedArray(shape, dtype))
            zero_outs.append(np.zeros(shape, dtype))
    n_params = len(in_names)
    n_outs = len(out_avals)
    all_names = list(in_names) + out_names + ([partition_name] if partition_name else [])

    def _body(*args):
        operands = list(args)
        if partition_name is not None:
            operands.append(partition_id_tensor())
        return tuple(_bass_exec_p.bind(
            *operands, out_avals=tuple(out_avals), in_names=tuple(all_names),
            out_names=tuple(out_names), lowering_input_output_aliases=(),
            sim_require_finite=True, sim_require_nnan=True, nc=nc))

    devices = jax.devices()[:NC_CORES]
    mesh = Mesh(np.array(devices), ("core",))
    donate = tuple(range(n_params, n_params + n_outs))
    sharded = jax.jit(
        shard_map(_body, mesh=mesh,
                  in_specs=(PartitionSpec("core"),) * (n_params + n_outs),
                  out_specs=(PartitionSpec("core"),) * n_outs,
                  check_rep=False),
        donate_argnums=donate, keep_unused=True)
    concat_in = [np.concatenate([np.asarray(in_maps[c][n]) for c in range(NC_CORES)], axis=0)
                 for n in in_names]
    dev_in = [jax.device_put(a) for a in concat_in]
    times = []
    out = None
    for it in range(iters):
        zeros = [jax.device_put(np.zeros((NC_CORES * z.shape[0], *z.shape[1:]), z.dtype))
                 for z in zero_outs]
        jax.block_until_ready(zeros)
        t0 = time.perf_counter()
        out = sharded(*dev_in, *zeros)
        jax.block_until_ready(out)
        times.append(time.perf_counter() - t0)
    res = np.asarray(out[0]).reshape(NC_CORES, COUT, T)
    full = np.concatenate([res[c].T.reshape(BL, L, COUT) for c in range(NC_CORES)], axis=0)
    return full, times


# revision 49
# speedup vs baseline: 1.1319x; 1.1319x over previous
"""AnomalyTransformer forward on 8 trn2 NeuronCores — pure data-parallel over batch.

Layout strategy (per core, 32 batches = 3200 tokens):
  - canonical activations: token-major bf16 tiles A[b] = [100 tok, 512 feat]
  - feature-major copies hT (via PE transpose) feed all weight matmuls
  - attention per batch of a group of 8: scores from feature-major q/k ->
    exp -> PE transpose of UNNORMALIZED e (no softmax-stat dependency) ->
    AV into one PSUM bank -> single fused normalize (PSUM read + 1/rowsum
    broadcast) -> transpose of o -> Wo as 4 full-contraction chunk matmuls
  - 5-stage software pipeline over batches; the DVE-gated pipeline tail of
    each group is emitted after the next group's QKV GEMMs (or c1/next
    layer's QKV at phase/layer boundaries) so the in-order PE never waits
    on the softmax/LN backlog
  - layer phases: all attention groups, then all FFN groups — keeps the
    activation-table (Exp vs Gelu) switches to 2 per layer
  - FFN: c1 produced feature-major (yT direct, no transpose back), c2 from
    yT chunks with the residual folded into PSUM via an identity matmul
    (z copy on the Act engine, LN stats on DVE)
  - LN: bn_stats/bn_aggr + rsqrt via bit-trick+Newton on DVE; affine
    scales/biases folded host-side; the final LN is fused into the last
    LN2 analytically (LN of a standardized vector => adjusted epsilon)
  - next layer's attention weights prefetched at phase-B start; FFN
    weights after phase B; projection per group, pipelined
"""

import sys
import numpy as np

for _p in ("/opt/trn_rl_repo",):
    if _p not in sys.path:
        sys.path.insert(0, _p)

import ml_dtypes
import concourse.bass as bass
import concourse.bacc as bacc_mod
import concourse.mybir as mybir
from concourse.tile import TileContext
from concourse.bass_utils import run_bass_kernel_spmd

BF16 = ml_dtypes.bfloat16

# model dims
B, L, CIN, COUT = 256, 100, 38, 38
D, H, E, DFF = 512, 8, 3, 512
DH = D // H
NC_CORES = 8
BL = B // NC_CORES          # 32 batches per core
T = BL * L                  # 3200 tokens per core
P = 128
GB = 8                      # batches per attention group
NG = BL // GB               # 4 groups
GT = GB * L                 # 800 tokens per group
KAUG = 3 * CIN + L          # 214 embed contraction rows

f32 = mybir.dt.float32
bf16 = mybir.dt.bfloat16
i32 = mybir.dt.int32
AF = mybir.ActivationFunctionType
ALU = mybir.AluOpType
AX = mybir.AxisListType

MAGIC_P1 = 0x5F3759DF + 1


def _ts(i, n=P):
    return slice(i * n, (i + 1) * n)


def build_nc(trivial_affine=True, zero_bias=True):
    nc = bacc_mod.Bacc()

    # ---- DRAM parameters ------------------------------------------------
    xaugT = nc.declare_dram_parameter("xaugT", [KAUG, T], bf16, isOutput=False)
    wcat = nc.declare_dram_parameter("wcat", [KAUG, D], bf16, isOutput=False)
    wqt = nc.declare_dram_parameter("wqt", [E, 4, P, D], bf16, isOutput=False)
    wkt = nc.declare_dram_parameter("wkt", [E, 4, P, D], bf16, isOutput=False)
    wvt = nc.declare_dram_parameter("wvt", [E, 4, P, D], bf16, isOutput=False)
    wocat = nc.declare_dram_parameter("wocat", [E, 4, P, D], bf16, isOutput=False)
    c1wt = nc.declare_dram_parameter("c1wt", [E, 4, P, D], bf16, isOutput=False)
    c2wt = nc.declare_dram_parameter("c2wt", [E, 4, P, D], bf16, isOutput=False)
    bqf = nc.declare_dram_parameter("bqf", [E, 4, P], f32, isOutput=False)
    bkf = nc.declare_dram_parameter("bkf", [E, 4, P], f32, isOutput=False)
    c1bf = nc.declare_dram_parameter("c1bf", [E, 4, P], f32, isOutput=False)
    btok = nc.declare_dram_parameter("btok", [E, 3, D], bf16, isOutput=False)
    lnsc = nc.declare_dram_parameter("lnsc", [E + 1, 2, 4, P], f32, isOutput=False)
    sbc = nc.declare_dram_parameter("sbc", [E, 2, P, D], bf16, isOutput=False)
    fbc = nc.declare_dram_parameter("fbc", [2, P, D], bf16, isOutput=False)
    maskb = nc.declare_dram_parameter("maskb", [L, 4 * L], bf16, isOutput=False)
    identd = nc.declare_dram_parameter("identd", [P, P], bf16, isOutput=False)
    projt = nc.declare_dram_parameter("projt", [4, P, COUT], bf16, isOutput=False)
    projb = nc.declare_dram_parameter("projb", [COUT], f32, isOutput=False)
    out_d = nc.declare_dram_parameter("out", [COUT, T], f32, isOutput=True)

    with TileContext(nc) as tc:
        with (
            tc.tile_pool(name="const", bufs=1) as cpool,
            tc.tile_pool(name="w", bufs=1) as wpool,
            tc.tile_pool(name="act", bufs=1) as apool,
            tc.tile_pool(name="yp", bufs=2) as ypool,
            tc.tile_pool(name="zs", bufs=9) as zpool,
            tc.tile_pool(name="grp", bufs=2) as gpool,
            tc.tile_pool(name="sc", bufs=3) as spool,
            tc.tile_pool(name="osb", bufs=2) as opool,
            tc.tile_pool(name="pmm", bufs=2, space="PSUM") as pmm,
            tc.tile_pool(name="psS", bufs=2, space="PSUM") as pS,
            tc.tile_pool(name="psT", bufs=2, space="PSUM") as pT,
            tc.tile_pool(name="psO", bufs=2, space="PSUM") as pO,
        ):
            # ---- embed inputs first: the first matmuls wait on these ---
            wc0e = wpool.tile([P, D], bf16, tag="wq0", name="wc0e")
            nc.sync.dma_start(out=wc0e[:, :], in_=wcat[0:P, :])
            wc1e = wpool.tile([KAUG - P, D], bf16, tag="wq1", name="wc1e")
            nc.sync.dma_start(out=wc1e[:, :], in_=wcat[P:KAUG, :])
            xa0e = apool.tile([P, T], bf16, tag="hT0", name="xa0e")
            xa1e = apool.tile([KAUG - P, T], bf16, tag="hT1", name="xa1e")
            for g in range(NG):
                gc = slice(g * GT, (g + 1) * GT)
                nc.sync.dma_start(out=xa0e[:, gc], in_=xaugT[0:P, gc])
                nc.sync.dma_start(out=xa1e[:, gc], in_=xaugT[P:KAUG, gc])

            # ---- constants ---------------------------------------------
            idt = cpool.tile([P, P], bf16, tag="ident", name="ident")
            nc.sync.dma_start(out=idt[:, :], in_=identd[:, :])
            mk = cpool.tile([L, 4 * L], bf16, tag="maskb", name="maskb")
            nc.sync.dma_start(out=mk[:, :], in_=maskb[:, :])
            ones1 = cpool.tile([1, P], bf16, tag="ones1", name="ones1")
            nc.vector.memset(ones1[:, :], 1.0)
            pjt = []
            for c in range(4):
                tl = cpool.tile([P, COUT], bf16, tag=f"pjt{c}", name=f"pjt{c}")
                nc.sync.dma_start(out=tl[:, :], in_=projt[c])
                pjt.append(tl)
            pjb = cpool.tile([COUT, 1], f32, tag="pjb", name="pjb")
            nc.sync.dma_start(out=pjb[:, :], in_=projb[:].unsqueeze(1))
            fs = cpool.tile([P, D], bf16, tag="fs", name="fs")
            nc.sync.dma_start(out=fs[:, :], in_=fbc[0])
            fb = cpool.tile([P, D], bf16, tag="fb", name="fb")
            nc.sync.dma_start(out=fb[:, :], in_=fbc[1])
            stats = cpool.tile([L, 2 * BL], f32, tag="stats", name="stats")

            # persistent activation buffers
            A = [apool.tile([L, D], bf16, tag=f"A{b}", name=f"A{b}") for b in range(BL)]
            hT = [apool.tile([P, T], bf16, tag=f"hT{c}", name=f"hT{c}") for c in range(4)]

            # ---- LN helpers --------------------------------------------
            def ln_rsqrt(b0, nb, final_eps=False):
                """rstd for batches [b0, b0+nb) from stats -> returns y tile."""
                w = spool.tile([L, GB], f32, tag="lnw", name="lnw")
                y = spool.tile([L, GB], f32, tag="lny", name="lny")
                t1 = spool.tile([L, GB], f32, tag="lnt", name="lnt")
                vs = stats[:, 2 * b0 + 1: 2 * (b0 + nb): 2]
                if final_eps:
                    # LN(LN(x)) with unit affine == LN with rstd
                    # 1/sqrt(v(1+eps) + eps^2): folds the final LN away
                    nc.vector.tensor_scalar(w[:, 0:nb], vs, 1.0 + 1e-5, 1e-10,
                                            op0=ALU.mult, op1=ALU.add)
                else:
                    nc.vector.tensor_scalar(w[:, 0:nb], vs, 1e-5, None,
                                            op0=ALU.add)
                wi = w[:, 0:nb].bitcast(i32)
                yi = y[:, 0:nb].bitcast(i32)
                ti = t1[:, 0:nb].bitcast(i32)
                nc.vector.tensor_scalar(ti, wi, 1, None,
                                        op0=ALU.logical_shift_right)
                nc.vector.tensor_scalar(ti, ti, -1, None, op0=ALU.bitwise_xor)
                nc.vector.tensor_scalar(yi, ti, MAGIC_P1, None, op0=ALU.add)
                for _ in range(2):
                    nc.vector.tensor_mul(t1[:, 0:nb], y[:, 0:nb], y[:, 0:nb])
                    nc.vector.tensor_mul(t1[:, 0:nb], t1[:, 0:nb], w[:, 0:nb])
                    nc.vector.tensor_scalar(t1[:, 0:nb], t1[:, 0:nb], -0.5, 1.5,
                                            op0=ALU.mult, op1=ALU.add)
                    nc.vector.tensor_mul(y[:, 0:nb], y[:, 0:nb], t1[:, 0:nb])
                return y

            def ln_stats(z, b):
                st6 = spool.tile([L, 6], f32, tag="st6", name="st6", bufs=6)
                nc.vector.bn_stats(st6[:, :], z[:, :])
                nc.vector.bn_aggr(stats[:, 2 * b:2 * b + 2], st6[:, :])

            def ln_apply(zs, b0, nb, final_eps=False):
                y = ln_rsqrt(b0, nb, final_eps)
                for j in range(nb):
                    b = b0 + j
                    eng = nc.gpsimd if j % 2 else nc.vector
                    eng.tensor_scalar(
                        A[b][:, :], zs[b][:, :],
                        stats[:, 2 * b:2 * b + 1], y[:, j:j + 1],
                        op0=ALU.subtract, op1=ALU.mult)

            def transpose_group(src_tiles, b0, nb, dst, scale_ch):
                """dst[c][:, (b0..b0+nb)*L] = src[b][:, chunk c].T (*scale)."""
                for c in range(4):
                    ps = pmm.tile([P, 2 * D], bf16, tag="mm", name="mm")
                    for j in range(nb):
                        nc.tensor.matmul(ps[:, j * L:(j + 1) * L],
                                         src_tiles[b0 + j][:, _ts(c)],
                                         idt[0:L, 0:L], is_transpose=True,
                                         start=(j == 0), stop=(j == nb - 1))
                    w = nb * L
                    dst_ap = dst[c][:, b0 * L:b0 * L + w]
                    if scale_ch is not None:
                        nc.scalar.activation(dst_ap, ps[:, 0:w], AF.Identity,
                                             bias=0.0,
                                             scale=scale_ch[c][:, 0:1])
                    else:
                        if c % 2 == 0:
                            nc.scalar.activation(dst_ap, ps[:, 0:w], AF.Identity)
                        else:
                            nc.vector.tensor_copy(dst_ap, ps[:, 0:w])

            # ---- embed: h0 = xaug @ wcat  (PE folded into wcat rows) ----
            wc0, wc1, xa0, xa1 = wc0e, wc1e, xa0e, xa1e
            for b in range(BL):
                bc = slice(b * L, (b + 1) * L)
                # alternate psum pools: 4 banks in flight keep PE ahead of
                # the copy engines
                pool, tg = (pmm, "mm") if b % 2 else (pS, "S")
                ps = pool.tile([L, D], f32, tag=tg, name="mm")
                nc.tensor.matmul(ps[:, :], xa0[:, bc], wc0[:, :],
                                 start=True, stop=False)
                nc.tensor.matmul(ps[:, :], xa1[:, bc], wc1[:, :],
                                 start=False, stop=True)
                if b % 2 == 0:
                    nc.vector.tensor_copy(A[b][:, :], ps[:, :])
                else:
                    nc.scalar.activation(A[b][:, :], ps[:, :], AF.Identity)
            # hT0/hT1 alias xa0/xa1 (tag ring); the slot handback requires all
            # xa reads emitted first, so transposes must follow the full embed
            for bg in range(BL // 4):
                transpose_group(A, bg * 4, 4, hT, None)

            # ---- attention group ---------------------------------------
            attn_st = {}

            def attn_qkv(g, Wq, Wk, Wv, bq, bk):
                """QKV GEMMs for a group; state saved for the step pipeline."""
                g0 = g * GT
                Qg = [gpool.tile([P, GT], bf16, tag=f"qg{c}", name=f"qg{c}")
                      for c in range(4)]
                Kg = [gpool.tile([P, GT], bf16, tag=f"kg{c}", name=f"kg{c}")
                      for c in range(4)]
                for co in range(4):
                    for hh in range(2):
                        colv = slice(g0 + hh * 400, g0 + (hh + 1) * 400)
                        ps = pmm.tile([P, D], f32, tag="mm", name="mm")
                        for ci in range(4):
                            nc.tensor.matmul(ps[:, 0:400],
                                             Wq[ci][:, _ts(co)],
                                             hT[ci][:, colv],
                                             start=(ci == 0), stop=(ci == 3))
                        if zero_bias:
                            nc.scalar.activation(
                                Qg[co][:, hh * 400:(hh + 1) * 400],
                                ps[:, 0:400], AF.Identity)
                        else:
                            nc.scalar.activation(
                                Qg[co][:, hh * 400:(hh + 1) * 400],
                                ps[:, 0:400], AF.Identity,
                                bias=bq[co][:, 0:1])
                        ps = pmm.tile([P, D], f32, tag="mm", name="mm")
                        for ci in range(4):
                            nc.tensor.matmul(ps[:, 0:400],
                                             Wk[ci][:, _ts(co)],
                                             hT[ci][:, colv],
                                             start=(ci == 0), stop=(ci == 3))
                        if zero_bias:
                            if co % 2:
                                nc.scalar.activation(
                                    Kg[co][:, hh * 400:(hh + 1) * 400],
                                    ps[:, 0:400], AF.Identity)
                            else:
                                nc.vector.tensor_copy(
                                    Kg[co][:, hh * 400:(hh + 1) * 400],
                                    ps[:, 0:400])
                        else:
                            nc.vector.tensor_scalar(
                                Kg[co][:, hh * 400:(hh + 1) * 400],
                                ps[:, 0:400], bk[co][:, 0:1], None,
                                op0=ALU.add)
                Qg2 = [gpool.tile([64, GT], bf16, tag=f"qh{c}", name=f"qh{c}",
                                  bufs=1)
                       for c in range(4)]
                Kg2 = [gpool.tile([64, GT], bf16, tag=f"kh{c}", name=f"kh{c}",
                                  bufs=1)
                       for c in range(4)]
                for c in range(4):
                    nc.sync.dma_start(out=Qg2[c][:, :], in_=Qg[c][64:128, :])
                    nc.sync.dma_start(out=Kg2[c][:, :], in_=Kg[c][64:128, :])
                Vg = [gpool.tile([L, D], bf16, tag=f"vg{b}", name=f"vg{b}")
                      for b in range(GB)]
                for b in range(GB):
                    ps = pmm.tile([L, D], f32, tag="mm", name="mm")
                    for ci in range(4):
                        nc.tensor.matmul(ps[:, :],
                                         hT[ci][:, g0 + b * L:g0 + (b + 1) * L],
                                         Wv[ci][:, :],
                                         start=(ci == 0), stop=(ci == 3))
                    if b % 2 == 0:
                        nc.vector.tensor_copy(Vg[b][:, :], ps[:, :])
                    else:
                        nc.scalar.activation(Vg[b][:, :], ps[:, :], AF.Identity)

                attn_st[g] = dict(Qg=Qg, Kg=Kg, Qg2=Qg2, Kg2=Kg2, Vg=Vg,
                                  e={}, r8={}, eT={}, oc={}, ocT={}, zs={})

            def attn_steps(g, Wo, bo, sinb, steps):
                """Run pipeline steps for group g (5 stages per batch)."""
                st = attn_st[g]
                b0 = g * GB
                Qg, Kg, Qg2, Kg2, Vg = (st["Qg"], st["Kg"], st["Qg2"],
                                        st["Kg2"], st["Vg"])
                e_t, r8_t, eT_t, oc_t, ocT_t, zs = (st["e"], st["r8"],
                                                    st["eT"], st["oc"],
                                                    st["ocT"], st["zs"])

                def s_scores(b):
                    """scores + mask + exp; row-sum reciprocal on DVE."""
                    e = spool.tile([L, H * L], bf16, tag="e", name="e", bufs=4)
                    for half in range(2):
                        psS = pS.tile([L, D], f32, tag="S", name="S")
                        for hj in range(4):
                            h = half * 4 + hj
                            c = h // 2
                            qsrc = Qg2[c] if h % 2 else Qg[c]
                            ksrc = Kg2[c] if h % 2 else Kg[c]
                            nc.tensor.matmul(
                                psS[:, hj * L:(hj + 1) * L],
                                qsrc[0:64, b * L:(b + 1) * L],
                                ksrc[0:64, b * L:(b + 1) * L],
                                start=(hj == 0), stop=False)
                        nc.tensor.matmul(psS[:, 0:4 * L], idt[0:L, 0:L],
                                         mk[:, :], start=False, stop=True)
                        nc.scalar.activation(
                            e[:, half * 4 * L:(half + 1) * 4 * L],
                            psS[:, 0:4 * L], AF.Exp)
                    s8 = spool.tile([L, H], f32, tag="s8", name="s8", bufs=4)
                    nc.vector.reduce_sum(
                        out=s8[:, :],
                        in_=e[:, :].rearrange("p (h m) -> p h m", h=H),
                        axis=AX.X)
                    r8 = spool.tile([L, H], f32, tag="r8", name="r8", bufs=4)
                    nc.vector.reciprocal(r8[:, :], s8[:, :])
                    e_t[b] = e
                    r8_t[b] = r8

                def s_etrans(b):
                    """PE transpose of UNNORMALIZED e (no softmax-stat dep)."""
                    e = e_t.pop(b)
                    eTb = spool.tile([L, H * L], bf16, tag="eT", name="eTb",
                                     bufs=4)
                    psA = pT.tile([L, 2 * D], bf16, tag="aTp", name="aTp")
                    for h in range(H):
                        nc.tensor.matmul(psA[:, h * L:(h + 1) * L],
                                         e[:, h * L:(h + 1) * L],
                                         idt[0:L, 0:L], is_transpose=True,
                                         start=(h == 0), stop=(h == 7))
                    nc.scalar.activation(eTb[:, :], psA[:, 0:8 * L],
                                         AF.Identity)
                    eT_t[b] = eTb

                def s_av(b):
                    """AV into one PSUM bank + fused normalize (DVE)."""
                    eTb = eT_t.pop(b)
                    psO = pO.tile([L, D], f32, tag="O", name="O")
                    for h in range(H):
                        nc.tensor.matmul(
                            psO[:, h * DH:(h + 1) * DH],
                            eTb[0:L, h * L:(h + 1) * L],
                            Vg[b][:, h * DH:(h + 1) * DH],
                            start=(h == 0), stop=(h == 7))
                    oc = spool.tile([L, D], bf16, tag="oc", name="oc", bufs=4)
                    r8 = r8_t.pop(b)
                    nc.vector.tensor_mul(
                        oc[:, :].rearrange("p (h d) -> p h d", h=H),
                        psO[:, :].rearrange("p (h d) -> p h d", h=H),
                        r8[:, :].to_broadcast((L, H, DH)))
                    oc_t[b] = oc

                def s_octrans(b):
                    """transpose oc into feature-major chunks."""
                    oc = oc_t.pop(b)
                    psB = pT.tile([P, 2 * D], bf16, tag="aTp", name="aTp")
                    for c in range(4):
                        nc.tensor.matmul(psB[:, c * L:(c + 1) * L],
                                         oc[:, _ts(c)],
                                         idt[0:L, 0:L], is_transpose=True,
                                         start=(c == 0), stop=(c == 3))
                    ocTb = spool.tile([P, 4 * L], bf16, tag="ocT", name="ocTb",
                                      bufs=4)
                    nc.scalar.activation(ocTb[:, :], psB[:, 0:4 * L],
                                         AF.Identity)
                    ocT_t[b] = ocTb

                def s_wo(b):
                    """Wo chunk matmuls, residual, ln stats."""
                    bg = b0 + b
                    ocTb = ocT_t.pop(b)
                    psZ = pmm.tile([L, D], f32, tag="mm", name="mm")
                    for c in range(4):
                        nc.tensor.matmul(psZ[:, :],
                                         ocTb[:, c * L:(c + 1) * L],
                                         Wo[c][:, :],
                                         start=(c == 0),
                                         stop=(c == 3 and zero_bias))
                    if not zero_bias:
                        nc.tensor.matmul(psZ[:, :], ones1[:, 0:L],
                                         bo[:, :], start=False, stop=True)
                    if trivial_affine:
                        rsrc = A[bg]
                    else:
                        rsrc = spool.tile([L, D], bf16, tag="r", name="r")
                        nc.gpsimd.tensor_mul(rsrc[:, :], A[bg][:, :],
                                             sinb[0:L, :])
                    z = zpool.tile([L, D], bf16, tag="za", name="z")
                    nc.vector.tensor_add(z[:, :], psZ[:, :], rsrc[:, :])
                    zs[bg] = z
                    ln_stats(z, bg)

                stages = (s_scores, s_etrans, s_av, s_octrans, s_wo)
                for step in steps:
                    for d, fn in enumerate(stages):
                        b = step - d
                        if 0 <= b < GB:
                            fn(b)
                return zs

            # ---- FFN pieces (c1 feature-major; c2 + LN2; T2 separate) --
            def c1_group(g, C1, c1b):
                g0 = g * GT
                yTg = [ypool.tile([P, GT], bf16, tag=f"yt{c}", name=f"yt{c}")
                       for c in range(4)]
                for co in range(4):
                    for hh in range(2):
                        colv = slice(g0 + hh * 400, g0 + (hh + 1) * 400)
                        ps = pmm.tile([P, D], f32, tag="mm", name="mm")
                        for ci in range(4):
                            nc.tensor.matmul(ps[:, 0:400],
                                             C1[ci][:, _ts(co)],
                                             hT[ci][:, colv],
                                             start=(ci == 0), stop=(ci == 3))
                        if zero_bias:
                            nc.scalar.activation(
                                yTg[co][:, hh * 400:(hh + 1) * 400],
                                ps[:, 0:400], AF.Gelu)
                        else:
                            nc.scalar.activation(
                                yTg[co][:, hh * 400:(hh + 1) * 400],
                                ps[:, 0:400], AF.Gelu,
                                bias=c1b[co][:, 0:1])
                return yTg

            def c2_half(g, half, yTg, C2, btf, s1b, fuse_final):
                b0 = g * GB
                zs = {}
                for j in range(4):
                    b = half * 4 + j
                    bg = b0 + b
                    if trivial_affine:
                        rsrc = A[bg]
                    else:
                        rsrc = spool.tile([L, D], bf16, tag="r", name="r")
                        nc.gpsimd.tensor_mul(rsrc[:, :], A[bg][:, :],
                                             s1b[0:L, :])
                    ps2 = pmm.tile([L, D], f32, tag="mm", name="mm")
                    for ci in range(4):
                        nc.tensor.matmul(ps2[:, :],
                                         yTg[ci][:, b * L:(b + 1) * L],
                                         C2[ci][:, :],
                                         start=(ci == 0), stop=False)
                    if not zero_bias:
                        nc.tensor.matmul(ps2[:, :], ones1[:, 0:L],
                                         btf[1][:, :], start=False, stop=False)
                    # residual folded into PSUM: ps2 += I @ A[bg]
                    nc.tensor.matmul(ps2[:, :], idt[0:L, 0:L], rsrc[:, :],
                                     start=False, stop=True)
                    z = zpool.tile([L, D], bf16, tag="zf", name="z",
                                   bufs=10)
                    nc.scalar.activation(z[:, :], ps2[:, :], AF.Identity)
                    zs[bg] = z
                    ln_stats(z, bg)
                ln_apply(zs, b0 + half * 4, 4, final_eps=fuse_final)

            def t2_group(g, scale_ch):
                transpose_group(A, g * GB, 4, hT, scale_ch)
                transpose_group(A, g * GB + 4, 4, hT, scale_ch)

            # ---- projection for one group ------------------------------
            def proj_group(g):
                for hh in range(2):
                    n0 = g * GT + hh * 400
                    ps = pmm.tile([P, D], f32, tag="mm", name="mm")
                    for ci in range(4):
                        nc.tensor.matmul(ps[0:COUT, 0:400], pjt[ci][:, :],
                                         hT[ci][:, n0:n0 + 400],
                                         start=(ci == 0), stop=(ci == 3))
                    osb = opool.tile([COUT, 400], f32, tag="osb", name="osb")
                    if zero_bias:
                        nc.vector.tensor_copy(osb[:, :], ps[0:COUT, 0:400])
                    else:
                        nc.vector.tensor_scalar(osb[:, :], ps[0:COUT, 0:400],
                                                pjb[:, 0:1], None, op0=ALU.add)
                    nc.sync.dma_start(out=out_d[:, n0:n0 + 400], in_=osb[:, :])

            # ---- final LN + projection for one group (non-trivial) -----
            def final_group(g):
                b0 = g * GB
                zs = {}
                for j in range(GB):
                    b = b0 + j
                    if trivial_affine:
                        z = A[b]
                    else:
                        r = spool.tile([L, D], bf16, tag="r", name="r")
                        nc.gpsimd.tensor_mul(r[:, :], A[b][:, :], fs[0:L, :])
                        z = zpool.tile([L, D], bf16, tag="zf", name="z",
                                       bufs=10)
                        nc.vector.tensor_add(z[:, :], r[:, :], fb[0:L, :])
                    zs[b] = z
                    ln_stats(z, b)
                ln_apply(zs, b0, GB)
                transpose_group(A, b0, 4, hT, None)
                transpose_group(A, b0 + 4, 4, hT, None)
                proj_group(g)

            # ---- layers -------------------------------------------------
            def load_w_attn(l):
                """Attention-side weights: last read in phase A, so they can
                be prefetched for l+1 at the start of layer l's phase B."""
                W = {}
                drms = dict(wq=wqt, wk=wkt, wv=wvt, wo=wocat)
                for nm in ("wq", "wk", "wv", "wo"):
                    W[nm] = []
                for c in range(4):
                    for nm in ("wq", "wk", "wv", "wo"):
                        tl = wpool.tile([P, D], bf16, tag=f"{nm}{c}",
                                        name=f"{nm}{c}")
                        nc.sync.dma_start(out=tl[:, :], in_=drms[nm][l, c])
                        W[nm].append(tl)
                for nm, drm in (("bq", bqf), ("bk", bkf)):
                    W[nm] = []
                    for c in range(4):
                        tl = wpool.tile([P, 1], f32, tag=f"{nm}{c}",
                                        name=f"{nm}{c}")
                        nc.sync.dma_start(out=tl[:, :],
                                          in_=drm[l, c].unsqueeze(1))
                        W[nm].append(tl)
                W["bo"] = wpool.tile([1, D], bf16, tag="bt0", name="bt0")
                nc.sync.dma_start(out=W["bo"][:, :], in_=btok[l, 0].unsqueeze(0))
                W["sinb"] = wpool.tile([P, D], bf16, tag="sinb", name="sinb")
                nc.sync.dma_start(out=W["sinb"][:, :], in_=sbc[l, 0])
                return W

            def load_w_ffn(l):
                """FFN-side weights: read throughout phase B, so the next
                layer's copies are loaded only after that loop is emitted."""
                W = {"c1": [], "c2": [], "cb": [], "s1": [], "sn": []}
                for c in range(4):
                    for nm, drm in (("c1", c1wt), ("c2", c2wt)):
                        tl = wpool.tile([P, D], bf16, tag=f"{nm}{c}",
                                        name=f"{nm}{c}")
                        nc.sync.dma_start(out=tl[:, :], in_=drm[l, c])
                        W[nm].append(tl)
                    tl = wpool.tile([P, 1], f32, tag=f"cb{c}", name=f"cb{c}")
                    nc.sync.dma_start(out=tl[:, :],
                                      in_=c1bf[l, c].unsqueeze(1))
                    W["cb"].append(tl)
                    tl = wpool.tile([P, 1], f32, tag=f"s1{c}", name=f"s1{c}")
                    nc.sync.dma_start(out=tl[:, :],
                                      in_=lnsc[l, 1, c].unsqueeze(1))
                    W["s1"].append(tl)
                    tl = wpool.tile([P, 1], f32, tag=f"sn{c}", name=f"sn{c}")
                    nc.sync.dma_start(out=tl[:, :],
                                      in_=lnsc[l + 1, 0, c].unsqueeze(1))
                    W["sn"].append(tl)
                W["btf"] = []
                for i in (1, 2):
                    tl = wpool.tile([1, D], bf16, tag=f"bt{i}", name=f"bt{i}")
                    nc.sync.dma_start(out=tl[:, :], in_=btok[l, i].unsqueeze(0))
                    W["btf"].append(tl)
                W["s1b"] = wpool.tile([P, D], bf16, tag="s1b", name="s1b")
                nc.sync.dma_start(out=W["s1b"][:, :], in_=sbc[l, 1])
                return W

            NS = 5
            Wc = load_w_attn(0)
            Wc.update(load_w_ffn(0))
            head0_done = False
            for l in range(E):
                last_layer = (l == E - 1)
                Wq, Wk, Wv, Wo = Wc["wq"], Wc["wk"], Wc["wv"], Wc["wo"]
                C1, C2 = Wc["c1"], Wc["c2"]
                bq, bk, c1b = Wc["bq"], Wc["bk"], Wc["cb"]
                s1_ch, sn_ch = Wc["s1"], Wc["sn"]
                bo, btf = Wc["bo"], Wc["btf"]
                sinb, s1b = Wc["sinb"], Wc["s1b"]

                fuse_final = last_layer and trivial_affine
                t1_sc = None if trivial_affine else s1_ch
                t2_sc = (None if (trivial_affine or last_layer) else sn_ch)

                # phase A: attention over all groups. The DVE-gated tail of
                # group g is emitted AFTER group g+1's QKV GEMMs (or c1 of
                # group 0 for the last group) so the PE never waits on the
                # softmax/LN backlog.
                yT_g = [None] * NG
                if not head0_done:
                    attn_qkv(0, Wq, Wk, Wv, bq, bk)
                    attn_steps(0, Wo, bo, sinb, range(GB))
                for g in range(NG):
                    if g + 1 < NG:
                        attn_qkv(g + 1, Wq, Wk, Wv, bq, bk)
                    else:
                        yT_g[0] = c1_group(0, C1, c1b)
                    zs_g = attn_steps(g, Wo, bo, sinb,
                                      range(GB, GB + NS - 1))
                    ln_apply(zs_g, g * GB, GB)
                    if g + 1 < NG:
                        attn_steps(g + 1, Wo, bo, sinb, range(GB))
                        transpose_group(A, g * GB, 4, hT, t1_sc)
                        transpose_group(A, g * GB + 4, 4, hT, t1_sc)
                    del attn_st[g]

                # phase B: FFN over all groups (single act-table switch);
                # T1 of the last group is hidden under c2h0 of group 0, the
                # next layer's weights are prefetched, and its QKV + first
                # head overlap this layer's LN2 tail.
                c2_half(0, 0, yT_g[0], C2, btf, s1b, fuse_final)
                transpose_group(A, (NG - 1) * GB, 4, hT, t1_sc)
                transpose_group(A, (NG - 1) * GB + 4, 4, hT, t1_sc)
                Wn = load_w_attn(l + 1) if not last_layer else None
                for g in range(NG):
                    if g > 0:
                        c2_half(g, 0, yT_g[g], C2, btf, s1b, fuse_final)
                    c2_half(g, 1, yT_g[g], C2, btf, s1b, fuse_final)
                    if g + 1 < NG:
                        yT_g[g + 1] = c1_group(g + 1, C1, c1b)
                        if not last_layer or fuse_final:
                            t2_group(g, None if fuse_final else t2_sc)
                        if last_layer:
                            if trivial_affine:
                                proj_group(g)
                            else:
                                final_group(g)
                    else:
                        if not last_layer:
                            attn_qkv(0, Wn["wq"], Wn["wk"], Wn["wv"],
                                     Wn["bq"], Wn["bk"])
                            t2_group(g, t2_sc)
                            attn_steps(0, Wn["wo"], Wn["bo"], Wn["sinb"],
                                       range(GB))
                            head0_done = True
                        elif fuse_final:
                            t2_group(g, None)
                            proj_group(g)
                        else:
                            final_group(g)
                if not last_layer:
                    Wn.update(load_w_ffn(l + 1))
                Wc = Wn

    nc.compile()
    return nc


# ---------------------------------------------------------------------------
# host side
# ---------------------------------------------------------------------------

def _pos_encoding():
    pos = np.arange(L)[:, None].astype(np.float32)
    div = np.exp(np.arange(0, D, 2).astype(np.float32) * (-np.log(10000.0) / D))
    pe = np.zeros((L, D), dtype=np.float32)
    pe[:, 0::2] = np.sin(pos * div)
    pe[:, 1::2] = np.cos(pos * div)
    return pe


def _chunk4(mT):
    """[D, N] -> [4, 128, N]"""
    return np.ascontiguousarray(mT.reshape(4, P, -1))


_NC = None
_NC_FLAGS = None


def _get_nc(trivial_affine=True, zero_bias=True):
    global _NC, _NC_FLAGS
    if _NC is None or _NC_FLAGS != (trivial_affine, zero_bias):
        _NC = build_nc(trivial_affine, zero_bias)
        _NC_FLAGS = (trivial_affine, zero_bias)
    return _NC


def is_trivial_affine(inputs):
    return (np.all(np.asarray(inputs["ln1s"]) == 1.0)
            and np.all(np.asarray(inputs["ln2s"]) == 1.0)
            and np.all(np.asarray(inputs["ln2b"])[E - 1] == 0.0))


def prepare_maps(inputs):
    inp = {k: np.asarray(v) for k, v in inputs.items()}
    x = inp["x"].astype(np.float32)
    emb_w = inp["emb_w"].astype(np.float32)
    mask = inp["mask"].astype(np.float32)

    Wq, bqa = inp["Wq"], inp["bq"]
    Wk, bka = inp["Wk"], inp["bk"]
    Wv, bva = inp["Wv"], inp["bv"]
    Wo, boa = inp["Wo"], inp["bo"]
    c1w, c1b = inp["c1w"], inp["c1b"]
    c2w, c2b = inp["c2w"], inp["c2b"]
    ln1s, ln1b = inp["ln1s"], inp["ln1b"]
    ln2s, ln2b = inp["ln2s"], inp["ln2b"]
    lnfs, lnfb = inp["lnfs"], inp["lnfb"]
    proj_w, proj_b = inp["proj_w"], inp["proj_b"]

    scale = 1.0 / np.sqrt(DH)

    wqt = np.stack([_chunk4(Wq[l].T * scale) for l in range(E)]).astype(BF16)
    wkt = np.stack([_chunk4(Wk[l].T) for l in range(E)]).astype(BF16)
    wvt = np.stack([_chunk4(Wv[l].T) for l in range(E)]).astype(BF16)
    wocat = np.stack([_chunk4(Wo[l].T) for l in range(E)]).astype(BF16)
    c1wt = np.stack([_chunk4(c1w[l].T) for l in range(E)]).astype(BF16)
    c2wt = np.stack([_chunk4(c2w[l].T) for l in range(E)]).astype(BF16)

    bqf = np.zeros((E, 4, P), np.float32)
    bkf = np.zeros((E, 4, P), np.float32)
    c1bf = np.zeros((E, 4, P), np.float32)
    btok = np.zeros((E, 3, D), np.float32)
    lnsc = np.zeros((E + 1, 2, 4, P), np.float32)
    lnsc[:, :] = 1.0
    sbc = np.zeros((E, 2, P, D), np.float32)
    for l in range(E):
        b_in = ln2b[l - 1] if l > 0 else np.zeros(D, np.float32)
        s_in = ln2s[l - 1] if l > 0 else np.ones(D, np.float32)
        bq_eff = (bqa[l] + b_in @ Wq[l].T) * scale
        bk_eff = bka[l] + b_in @ Wk[l].T
        bv_eff = bva[l] + b_in @ Wv[l].T
        bo_eff = boa[l] + bv_eff @ Wo[l].T + b_in
        c1b_eff = c1b[l] + ln1b[l] @ c1w[l].T
        c2b_eff = c2b[l] + ln1b[l]
        bqf[l] = bq_eff.reshape(4, P)
        bkf[l] = bk_eff.reshape(4, P)
        c1bf[l] = c1b_eff.reshape(4, P)
        btok[l, 0] = bo_eff
        btok[l, 1] = c1b_eff
        btok[l, 2] = c2b_eff
        lnsc[l, 0] = s_in.reshape(4, P)
        lnsc[l, 1] = ln1s[l].reshape(4, P)
        sbc[l, 0] = np.tile(s_in[None, :], (P, 1))
        sbc[l, 1] = np.tile(ln1s[l][None, :], (P, 1))

    fbc = np.stack([np.tile(ln2s[E - 1][None, :], (P, 1)),
                    np.tile(ln2b[E - 1][None, :], (P, 1))]).astype(BF16)

    projw_eff = proj_w * lnfs[None, :]
    projb_eff = proj_b + lnfb @ proj_w.T
    projt = np.ascontiguousarray(
        projw_eff.T.reshape(4, P, COUT)).astype(BF16)

    maskb_np = np.tile(-30.0 * (1.0 - mask), (1, 4)).astype(BF16)
    ident = np.eye(P, dtype=np.float32).astype(BF16)
    wcat = np.concatenate([emb_w[:, :, 0].T, emb_w[:, :, 1].T,
                           emb_w[:, :, 2].T, _pos_encoding()], axis=0)

    shared = dict(
        wcat=wcat.astype(BF16), wqt=wqt, wkt=wkt, wvt=wvt, wocat=wocat,
        c1wt=c1wt, c2wt=c2wt, bqf=bqf, bkf=bkf, c1bf=c1bf,
        btok=btok.astype(BF16), lnsc=lnsc, sbc=sbc.astype(BF16),
        fbc=fbc, maskb=maskb_np, identd=ident, projt=projt,
        projb=projb_eff.astype(np.float32),
    )

    # per-core augmented input, feature-major [214, 3200]
    oh = np.eye(L, dtype=np.float32)
    in_maps = []
    for ci in range(NC_CORES):
        xs = x[ci * BL:(ci + 1) * BL]                      # [32, 100, 38]
        xp = np.concatenate([xs[:, -1:], xs, xs[:, :1]], axis=1)  # [32,102,38]
        feats = [xp[:, w:w + L, :] for w in range(3)]      # each [32,100,38]
        ohb = np.broadcast_to(oh[None], (BL, L, L))
        xaug = np.concatenate(feats + [ohb], axis=2)       # [32,100,214]
        xaugT = np.ascontiguousarray(
            xaug.reshape(T, KAUG).T).astype(BF16)          # [214, 3200]
        m = dict(shared)
        m["xaugT"] = xaugT
        in_maps.append(m)
    return in_maps


def is_zero_bias(inputs):
    i = {k: np.asarray(v) for k, v in inputs.items()}
    zb = True
    for l in range(E):
        b_in = i["ln2b"][l - 1] if l > 0 else np.zeros(D, np.float32)
        zb &= bool(np.all(i["bq"][l] + b_in @ i["Wq"][l].T == 0))
        zb &= bool(np.all(i["bk"][l] + b_in @ i["Wk"][l].T == 0))
        bv_eff = i["bv"][l] + b_in @ i["Wv"][l].T
        zb &= bool(np.all(i["bo"][l] + bv_eff @ i["Wo"][l].T + b_in == 0))
        zb &= bool(np.all(i["c1b"][l] + i["ln1b"][l] @ i["c1w"][l].T == 0))
        zb &= bool(np.all(i["c2b"][l] + i["ln1b"][l] == 0))
    zb &= bool(np.all(i["proj_b"] + i["lnfb"] @ i["proj_w"].T == 0))
    return zb


def run(inputs, **kw):
    nc = _get_nc(is_trivial_affine(inputs), is_zero_bias(inputs))
    in_maps = prepare_maps(inputs)
    res = run_bass_kernel_spmd(nc, in_maps, core_ids=list(range(NC_CORES)), **kw)
    outs = []
    for ci in range(NC_CORES):
        o = np.asarray(res.results[ci]["out"], np.float32)  # [38, 3200]
        outs.append(o.T.reshape(BL, L, COUT))
    full = np.concatenate(outs, axis=0)
    return full, res


def kernel(**inputs):
    full, _ = run(inputs)
    return full.astype(np.float32)


def bench(inputs, iters=6):
    """Steady-state wall timing of the sharded jitted executable."""
    import time
    import jax
    from jax.sharding import Mesh, PartitionSpec
    from jax.experimental.shard_map import shard_map
    from concourse import bass2jax, mybir
    from concourse.bass2jax import _bass_exec_p, install_neuronx_cc_hook, partition_id_tensor

    nc = _get_nc(is_trivial_affine(inputs), is_zero_bias(inputs))
    in_maps = prepare_maps(inputs)
    install_neuronx_cc_hook()
    partition_name = nc.partition_id_tensor.name if nc.partition_id_tensor else None
    in_names, out_names, out_avals, zero_outs = [], [], [], []
    for alloc in nc.m.functions[0].allocations:
        if not isinstance(alloc, mybir.MemoryLocationSet):
            continue
        name = alloc.memorylocations[0].name
        if alloc.kind == "ExternalInput":
            if name != partition_name:
                in_names.append(name)
        elif alloc.kind == "ExternalOutput":
            out_names.append(name)
            shape = tuple(alloc.tensor_shape)
            dtype = mybir.dt.np(alloc.dtype)
            out_avals.append(jax.core.ShapedArray(shape, dtype))
            zero_outs.append(np.zeros(shape, dtype))
    n_params = len(in_names)
    n_outs = len(out_avals)
    all_names = list(in_names) + out_names + ([partition_name] if partition_name else [])

    def _body(*args):
        operands = list(args)
        if partition_name is not None:
            operands.append(partition_id_tensor())
        return tuple(_bass_exec_p.bind(
            *operands, out_avals=tuple(out_avals), in_names=tuple(all_names),
            out_names=tuple(out_names), lowering_input_output_aliases=(),
            sim_require_finite=True, sim_require_nnan=True, nc=nc))

    devices = jax.devices()[:NC_CORES]
    mesh = Mesh(np.array(devices), ("core",))
    donate = tuple(range(n_params, n_params + n_outs))
    sharded = jax.jit(
        shard_map(_body, mesh=mesh,
                  in_specs=(PartitionSpec("core"),) * (n_params + n_outs),
                  out_specs=(PartitionSpec("core"),) * n_outs,
                  check_rep=False),
        donate_argnums=donate, keep_unused=True)
    concat_in = [np.concatenate([np.asarray(in_maps[c][n]) for c in range(NC_CORES)], axis=0)
                 for n in in_names]
    dev_in = [jax.device_put(a) for a in concat_in]
    times = []
    out = None
    for it in range(iters):
        zeros = [jax.device_put(np.zeros((NC_CORES * z.shape[0], *z.shape[1:]), z.dtype))
                 for z in zero_outs]
        jax.block_until_ready(zeros)
        t0 = time.perf_counter()
        out = sharded(*dev_in, *zeros)
        jax.block_until_ready(out)
        times.append(time.perf_counter() - t0)
    res = np.asarray(out[0]).reshape(NC_CORES, COUT, T)
    full = np.concatenate([res[c].T.reshape(BL, L, COUT) for c in range(NC_CORES)], axis=0)
    return full, times
